# revision 22
# baseline (speedup 1.0000x reference)
"""Causal dot-product attention (returns (o, a)) as a Trainium2 Bass kernel.

Problem: q,k,v [4,16,2048,64] fp32, causal mask m [1,1,2048,2048].
reference: a = softmax(mask(q@k^T/8)), o = a@v ; returns (o, a).

Sharding: B*H = 64 (b,h) pairs split across 8 cores, 8 pairs per core.
No cross-core communication.

Per-core per-pair plan (S=2048, D=64, P=128 partition tiles):
  - load q,k,v naturally; build qT,kT [64,S] via TensorE transposes.
  - pass T (transposed scores, feeds P@V):
      for each k-tile kt: scoresT[k=part, q=free] = kT_tile^T-mm, causal
      q-range only; mask diag block; exp (ACT, scale=1/8) -> ptT in SBUF;
      accumulate o^T[64, q] += V_tile^T-mm over kt with PSUM start/stop.
  - pass N (natural scores, feeds `a`):
      for each q-tile i: scores[q=part, k<= (i+1)*128] ; mask diag block;
      exp with accum_out => row sums l; rl = 1/l (DVE);
      a_tile = exp * rl (tensor_scalar) -> DMA out; zero-fill k>q region.
  - o = transpose(o^T) * rl -> DMA out.

Matmuls run in float32r (full-rate fp32 PE mode; plain fp32 is 4x slower).
"""

import os
import sys

import numpy as np

for _p in ("/opt/trn_rl_repo", os.path.expanduser("~/.axon_site/_ro/trn_rl_repo")):
    if os.path.isdir(_p) and _p not in sys.path:
        sys.path.insert(0, _p)

import concourse.bass as bass  # noqa: E402
import concourse.bacc as bacc  # noqa: E402
import concourse.mybir as mybir  # noqa: E402
import concourse.tile as tile  # noqa: E402
from concourse import bass_utils  # noqa: E402
from concourse.masks import make_causal_mask, make_identity  # noqa: E402

F32 = mybir.dt.float32
F32R = mybir.dt.float32r
EXP = mybir.ActivationFunctionType.Exp

B, H, S, D = 4, 16, 2048, 64
N_CORES = 8
PAIRS = (B * H) // N_CORES  # 8 (b,h) pairs per core
P = 128
NEG = -1e9
SCALE = 0.125  # 1/sqrt(D)


def _build_nc(S_=S, NP=PAIRS, mm=F32R, cap_waits=True):
    """Build the per-core Bass module (same program on all 8 cores).

    mm: matmul operand dtype (float32r = full-rate PE fp32 mode; float32 is
    4x slower but exact; bfloat16 also full-rate, lower precision). The
    producers of every matmul operand write this dtype directly (BIR
    verifier requires fp32r inputs to be rounded by their producer).
    """
    from contextlib import ExitStack

    NT = S_ // P  # number of 128-row tiles
    CH = min(1024, S_)  # PSUM score-chunk width (2 banks)
    JB = min(512, S_)  # P@V q-block width (1 PSUM bank)
    NJ = S_ // JB

    nc = bacc.Bacc()
    q_in = nc.dram_tensor("q_in", [NP, S_, D], F32, kind="ExternalInput")
    k_in = nc.dram_tensor("k_in", [NP, S_, D], F32, kind="ExternalInput")
    v_in = nc.dram_tensor("v_in", [NP, S_, D], F32, kind="ExternalInput")
    a_out = nc.dram_tensor("a_out", [NP, S_, S_], F32, kind="ExternalOutput")
    o_out = nc.dram_tensor("o_out", [NP, S_, D], F32, kind="ExternalOutput")

    with tile.TileContext(nc) as tc, ExitStack() as ctx:
        consts = ctx.enter_context(tc.tile_pool(name="consts", bufs=1))
        io = ctx.enter_context(tc.tile_pool(name="io", bufs=3))
        work = ctx.enter_context(tc.tile_pool(name="work", bufs=5))
        small = ctx.enter_context(tc.tile_pool(name="small", bufs=8))
        # phase-private PSUM pools so pass T of pair p+1 never waits on pass
        # N of pair p through FIFO slot rotation.  Bank budget (8 total):
        # scpT 2x[128,512]=2, ovp 2x[64,512]=2, scpN 2x[128,1024]=4.
        scpT = ctx.enter_context(tc.tile_pool(name="scpT", bufs=2, space="PSUM"))
        ovp = ctx.enter_context(tc.tile_pool(name="ovp", bufs=2, space="PSUM"))
        scpN = ctx.enter_context(tc.tile_pool(name="scpN", bufs=2, space="PSUM"))

        ident = consts.tile([P, P], F32)
        make_identity(nc, ident)
        # mask_n[x, y] = 0 if y <= x else NEG   (natural: q=part, k=free)
        mask_n = consts.tile([P, P], F32)
        make_causal_mask(nc, mask_n, mask_val=NEG)
        # mask_t[x, y] = 0 if y >= x else NEG   (transposed: k=part, q=free)
        mask_t = consts.tile([P, P], F32)
        nc.gpsimd.memset(mask_t, 0.0)
        nc.gpsimd.affine_select(
            out=mask_t,
            in_=mask_t,
            compare_op=mybir.AluOpType.is_ge,
            fill=NEG,
            base=0,
            pattern=[[1, P]],
            channel_multiplier=-1,
        )
        zeros = consts.tile([P, S_], F32)
        nc.gpsimd.memset(zeros, 0.0)

        for p in range(NP):
            # ---- load inputs, build qT/kT via PE transposes ----
            qn = io.tile([P, NT, D], F32, tag="qn")
            kn = io.tile([P, NT, D], F32, tag="kn")
            vn = io.tile([P, NT, D], F32, tag="vn")
            nc.sync.dma_start(out=qn, in_=q_in[p].rearrange("(t p) d -> p t d", p=P))
            nc.sync.dma_start(out=kn, in_=k_in[p].rearrange("(t p) d -> p t d", p=P))
            nc.sync.dma_start(out=vn, in_=v_in[p].rearrange("(t p) d -> p t d", p=P))
            if mm != F32:
                vr = io.tile([P, NT, D], mm, tag="vr")
                nc.vector.tensor_copy(vr, vn)
            else:
                vr = vn
            qT = io.tile([D, S_], mm, tag="qT")
            kT = io.tile([D, S_], mm, tag="kT")
            for src, dst in ((qn, qT), (kn, kT)):
                for g in range(0, NT, 4):
                    n = min(4, NT - g)
                    tp = scpT.tile([D, JB], F32, tag="scT")
                    for j in range(n):
                        nc.tensor.transpose(
                            tp[:, j * P : (j + 1) * P], src[:, g + j, :], ident
                        )
                    nc.vector.tensor_copy(
                        dst[:, g * P : (g + n) * P], tp[:, : n * P]
                    )

            # ---- pass T (J-outer): scoresT + exp + P@V per q-block ----
            KPB = JB // P  # k-tiles per q-block
            oT_sb = work.tile([D, S_], F32, tag="oT")
            for J in range(NJ):
                jlo = J * JB
                o_t = ovp.tile([D, JB], F32, tag="o")
                nkt = min(NT, (J + 1) * KPB)  # causal: k-tiles 0..nkt-1
                for kt in range(nkt):
                    qlo = kt * P
                    lo = max(qlo, jlo)  # first valid q col in this block
                    st = scpT.tile([P, JB], F32, tag="scT")
                    nc.tensor.matmul(
                        st[:, lo - jlo :],
                        lhsT=kT[:, qlo : qlo + P],
                        rhs=qT[:, lo : jlo + JB],
                        start=True,
                        stop=True,
                    )
                    if qlo >= jlo:  # diagonal block inside this q-block
                        nc.vector.tensor_add(
                            st[:, qlo - jlo : qlo - jlo + P],
                            st[:, qlo - jlo : qlo - jlo + P],
                            mask_t,
                        )
                    pt = work.tile([P, JB], mm, tag="pt")
                    if lo > jlo:
                        # memset rejects float32r; zero the raw bits instead
                        nc.vector.memset(
                            pt[:, : lo - jlo].bitcast(mybir.dt.uint32), 0
                        )
                    nc.scalar.activation(
                        out=pt[:, lo - jlo :],
                        in_=st[:, lo - jlo :],
                        func=EXP,
                        scale=SCALE,
                    )
                    nc.tensor.matmul(
                        o_t,
                        lhsT=vr[:, kt, :],
                        rhs=pt,
                        start=(kt == 0),
                        stop=(kt == nkt - 1),
                    )
                nc.vector.tensor_copy(oT_sb[:, jlo : jlo + JB], o_t)

            # ---- pass N: natural scores, exp+rowsum, normalize, stream a ----
            rl = small.tile([P, NT], F32, tag="rl")
            for i in range(NT):
                kw = (i + 1) * P
                nch = (kw + CH - 1) // CH
                lparts = []
                a_tiles = []
                for ch in range(nch):
                    clo = ch * CH
                    chi = min(kw, clo + CH)
                    sn = scpN.tile([P, CH], F32, tag="scN")
                    c = clo
                    while c < chi:
                        c1 = min((c // JB + 1) * JB, chi)
                        nc.tensor.matmul(
                            sn[:, c - clo : c1 - clo],
                            lhsT=qT[:, i * P : (i + 1) * P],
                            rhs=kT[:, c:c1],
                            start=True,
                            stop=True,
                        )
                        c = c1
                    dlo = i * P  # diagonal block columns [dlo, kw)
                    if clo <= dlo < chi:
                        nc.vector.tensor_add(
                            sn[:, dlo - clo : dlo - clo + P],
                            sn[:, dlo - clo : dlo - clo + P],
                            mask_n,
                        )
                    asb = work.tile([P, CH], F32, tag="asb")
                    lp = small.tile([P, 1], F32, tag="lp")
                    nc.scalar.activation(
                        out=asb[:, : chi - clo],
                        in_=sn[:, : chi - clo],
                        func=EXP,
                        scale=SCALE,
                        accum_out=lp,
                    )
                    lparts.append(lp)
                    a_tiles.append((asb, clo, chi))
                if nch == 1:
                    nc.vector.reciprocal(rl[:, i : i + 1], lparts[0])
                else:
                    lsum = small.tile([P, 1], F32, tag="lsumi")
                    nc.vector.tensor_add(lsum, lparts[0], lparts[1])
                    nc.vector.reciprocal(rl[:, i : i + 1], lsum)
                for asb, clo, chi in a_tiles:
                    nc.vector.tensor_scalar_mul(
                        asb[:, : chi - clo], asb[:, : chi - clo], rl[:, i : i + 1]
                    )
                    nc.sync.dma_start(
                        out=a_out[p, i * P : (i + 1) * P, clo:chi],
                        in_=asb[:, : chi - clo],
                    )
                if kw < S_:
                    nc.sync.dma_start(
                        out=a_out[p, i * P : (i + 1) * P, kw:],
                        in_=zeros[:, : S_ - kw],
                    )

            # ---- finalize o: transpose o^T, scale rows by rl, DMA out ----
            otr = scpN.tile([P, NT * D], F32, tag="scN")
            for t in range(NT):
                nc.tensor.transpose(
                    otr[:, t * D : (t + 1) * D], oT_sb[:, t * P : (t + 1) * P],
                    ident[:D, :D],
                )
            o_sb = work.tile([P, NT, D], F32, tag="osb")
            for t in range(NT):
                nc.vector.tensor_scalar_mul(
                    o_sb[:, t, :], otr[:, t * D : (t + 1) * D], rl[:, t : t + 1]
                )
            nc.sync.dma_start(
                out=o_out[p].rearrange("(t p) d -> p t d", p=P), in_=o_sb
            )

    if cap_waits:
        # bacc lowering (move_matmul_waits_to_ldweights, event semaphores,
        # reg alloc) -- required for walrus' per-instruction sync-wait limits.
        nc.compile()
    return nc


_NC_CACHE = {}
LAST_RESULTS = None  # BassKernelResults of the most recent device run


def _get_nc():
    if "nc" not in _NC_CACHE:
        _NC_CACHE["nc"] = _build_nc()
    return _NC_CACHE["nc"]


def _numpy_fallback(q, k, v, m):
    """Correct-for-any-mask host path (only used if mask is not causal)."""
    q64 = q.astype(np.float32)
    scores = np.einsum("bhqd,bhkd->bhqk", q64, k.astype(np.float32)) / np.sqrt(
        np.float32(q.shape[-1])
    )
    scores = np.where(m, -np.inf, scores)
    smax = np.max(scores, axis=-1, keepdims=True)
    e = np.exp(scores - smax)
    a = e / np.sum(e, axis=-1, keepdims=True)
    o = np.einsum("bhqk,bhkd->bhqd", a.astype(np.float32), v.astype(np.float32))
    return o.astype(np.float32), a.astype(np.float32)


def kernel(q, k, v, m):
    q = np.ascontiguousarray(q, dtype=np.float32)
    k = np.ascontiguousarray(k, dtype=np.float32)
    v = np.ascontiguousarray(v, dtype=np.float32)
    m_arr = np.asarray(m).reshape(np.asarray(m).shape[-2], np.asarray(m).shape[-1])
    causal = np.triu(np.ones((S, S), dtype=bool), k=1)
    if m_arr.shape != (S, S) or not np.array_equal(m_arr, causal):
        return _numpy_fallback(q, k, v, np.asarray(m))

    nc = _get_nc()
    qf = q.reshape(B * H, S, D)
    kf = k.reshape(B * H, S, D)
    vf = v.reshape(B * H, S, D)
    in_maps = [
        {
            "q_in": np.ascontiguousarray(qf[c * PAIRS : (c + 1) * PAIRS]),
            "k_in": np.ascontiguousarray(kf[c * PAIRS : (c + 1) * PAIRS]),
            "v_in": np.ascontiguousarray(vf[c * PAIRS : (c + 1) * PAIRS]),
        }
        for c in range(N_CORES)
    ]
    res = bass_utils.run_bass_kernel_spmd(nc, in_maps, core_ids=list(range(N_CORES)))
    global LAST_RESULTS
    LAST_RESULTS = res
    o = np.concatenate([r["o_out"] for r in res.results]).reshape(B, H, S, D)
    a = np.concatenate([r["a_out"] for r in res.results]).reshape(B, H, S, S)
    return o.astype(np.float32, copy=False), a.astype(np.float32, copy=False)
